# revision 13
# baseline (speedup 1.0000x reference)
"""Contrastive (NT-Xent) loss kernel for Trainium2, 8 NeuronCores SPMD.

Math (B=4096, D=256, T=0.5):
  z = l2norm(emb) rows; reps=[z_i; z_j] (8192 x 256); sim = reps @ reps.T
  denom_r = sum_{c != r} exp(sim[r,c]/T);  pos_m = z_i[m].z_j[m]
  loss = mean_r( ln(denom_r) - pos_r/T )

Distribution: core k receives ONLY its row shard x = [emb_i rows
[512k,512k+512); emb_j rows [512k,512k+512)] as fp16 (512KB/core instead
of a replicated 9MB) — H2D over the axon tunnel is the wall-clock
bottleneck, not device compute. Each core normalizes its 1024 rows,
transposes them to d-major fp16 tiles, and the 8 cores AllGather those
tiles HBM->HBM (512KB -> 4MB, ~15us on-chip, compute engines stay free).
The gathered column order is a core-major permutation of the reference
row order, which is harmless: the denominator is a permutation-invariant
row sum and the diagonal term is removed analytically (exp(2*||z||^2)=e^2).

Per-core main loop (8 m-tiles x 4 column groups of 2048):
  matmul fp16 -> PSUM fp32 [128,2048], ACT Exp(scale=2) in-place with
  accum_out -> per-(m,g) row partial sums; tail: ln(rowsum - e^2) minus
  4*sum(pos) -> per-partition partial [128,1] per core.
Host: loss = sum(partials)/(2B).  (gather/unshard = sum of shards)
"""

import numpy as np
from contextlib import ExitStack

import concourse.bass as bass
import concourse.tile as tile
from concourse import bacc, mybir
from concourse import bass_utils

B = 4096
D = 256
TEMP = 0.5
NCORES = 8
ROWS = 2 * B            # 8192 reps rows
PER = B // NCORES       # 512 rows of emb_i (and emb_j) per core
OWN = 2 * PER           # 1024 reps rows per core
P = 128
NG = 4                  # column groups
GCOLS = ROWS // NG      # 2048 columns per group
MT = OWN // P           # 8 m-tiles per core
NT = OWN // P           # 8 own row-tiles
F32 = mybir.dt.float32
DT = mybir.dt.float16   # matmul dtype
DTIN = mybir.dt.float8e4  # wire dtype (e4m3): halves H2D vs fp16; loss
                          # averages the ~6e-2 per-element quantization
                          # noise over 8192 rows x 8192 cols to ~1e-4
INV_T = 1.0 / TEMP      # 2.0
DIAG = float(np.exp(np.float32(INV_T), dtype=np.float32))  # exp(2*||z||^2), ||z||~1


def _kernel_body(ctx: ExitStack, tc: tile.TileContext, out_ap, x):
    nc = tc.nc
    AF = mybir.ActivationFunctionType
    ALU = mybir.AluOpType

    own_pool = ctx.enter_context(tc.tile_pool(name="own", bufs=1))
    zt_pool = ctx.enter_context(tc.tile_pool(name="zt", bufs=1))
    fin_pool = ctx.enter_context(tc.tile_pool(name="fin", bufs=1))
    ps_pool = ctx.enter_context(tc.tile_pool(name="ps", bufs=2, space="PSUM"))
    dram_pool = ctx.enter_context(tc.tile_pool(name="dram", bufs=1, space="DRAM"))

    rowparts = fin_pool.tile([P, MT * NG], F32, tag="rowparts")
    negdiag = fin_pool.tile([P, 1], F32, tag="negdiag")
    nc.gpsimd.memset(negdiag[:], -DIAG)

    # ---------------- own-block prologue ----------------
    # per-128-row-tile pipeline: load t -> cast t -> sq t -> reduce t, so
    # the norm chain streams behind the DMA instead of waiting for the
    # whole 256KB strided load (the collective trigger is downstream of
    # all of this, and every core's trigger time gates the rendezvous)
    # row mapping: SBUF (p, t) holds shard row 4p + t%4 of shard t//4
    # (i for t<4, j for t>=4). 1KB-contiguous per partition per DMA —
    # 4x fewer, 4x bigger descriptors than a row%128 layout. Any row
    # permutation is fine: the denominator sums over all columns, and
    # the i/j positive pairing (t <-> t+4) stays partition-aligned.
    nt2 = NT // 2
    own_x8 = own_pool.tile([P, NT, D], DTIN, tag="own_x8")  # [128,8,256]
    own_x = own_pool.tile([P, NT, D], DT, tag="own_x")
    sq3 = own_pool.tile([P, NT, D], F32, tag="sq3")
    sqs = own_pool.tile([P, NT], F32, tag="sqs")
    nc.sync.dma_start(own_x8[:, 0:nt2, :],
                      x[0:PER].rearrange("(p u) d -> p u d", u=nt2))
    nc.scalar.dma_start(own_x8[:, nt2:NT, :],
                        x[PER:].rearrange("(p u) d -> p u d", u=nt2))
    for t in range(NT):
        nc.vector.tensor_copy(own_x[:, t, :], own_x8[:, t, :])
        nc.vector.tensor_mul(sq3[:, t, :], own_x[:, t, :], own_x[:, t, :])
        nc.vector.reduce_sum(out=sqs[:, t:t + 1], in_=sq3[:, t, :],
                             axis=mybir.AxisListType.X)
    inv = own_pool.tile([P, NT], F32, tag="inv")
    nc.scalar.activation(out=inv[:], in_=sqs[:], func=AF.Ln)
    nc.scalar.activation(out=inv[:], in_=inv[:], func=AF.Exp, scale=-0.5)

    z_own = own_pool.tile([P, NT, D], DT, tag="z_own")
    for t in range(NT):
        nc.vector.tensor_scalar_mul(
            out=z_own[:, t, :], in0=own_x[:, t, :], scalar1=inv[:, t:t + 1])

    # transpose own rows to d-major: zt_own[h][d, col] with d in half h.
    # fp16 transposes (xbar needs 2-byte) alternating over both HWDGE
    # queues (SP + ACT), then a DVE cast to fp8 per half feeds the
    # collective with half the wire bytes; matmul also runs fp8 (2x PE).
    zt_own = [own_pool.tile([P, OWN], DT, tag=f"zt_own{h}", name=f"zt_own{h}")
              for h in range(2)]
    zt_own8 = [own_pool.tile([P, OWN], DTIN, tag=f"zt_own8{h}",
                             name=f"zt_own8{h}") for h in range(2)]
    cc_in = dram_pool.tile([2, P, OWN], DTIN, name="cc_in")
    for h in range(2):
        for t in range(NT):
            eng = nc.sync if t % 2 == 0 else nc.scalar
            eng.dma_start_transpose(
                out=zt_own[h][:, t * P:(t + 1) * P],
                in_=z_own[:, t, h * P:(h + 1) * P])
            # per-block fp8 cast pipelines behind its transpose instead of
            # one whole-half cast serializing after the last one
            nc.vector.tensor_copy(zt_own8[h][:, t * P:(t + 1) * P],
                                  zt_own[h][:, t * P:(t + 1) * P])
        nc.gpsimd.dma_start(cc_in[h], zt_own8[h][:])

    # ---------------- all-gather d-major z (fp8) ----------------
    cc_out = dram_pool.tile([NCORES, 2, P, OWN], DTIN, addr_space="Shared",
                            name="cc_out")
    nc.gpsimd.collective_compute(
        "AllGather", mybir.AluOpType.bypass,
        replica_groups=[list(range(NCORES))],
        ins=[cc_in.opt()], outs=[cc_out.opt()])

    # positives: pos_t = (x_i[t] . x_j[t]) * inv_i[t] * inv_j[t]
    # (issued after the collective trigger so DVE work hides in its shadow)
    nt2 = NT // 2
    pr3 = own_pool.tile([P, nt2, D], F32, tag="pr3")
    nc.vector.tensor_mul(pr3[:], own_x[:, 0:nt2, :], own_x[:, nt2:NT, :])
    pos = own_pool.tile([P, nt2], F32, tag="pos")
    nc.vector.reduce_sum(out=pos[:], in_=pr3[:], axis=mybir.AxisListType.X)
    nc.vector.tensor_mul(pos[:], pos[:], inv[:, 0:nt2])
    nc.vector.tensor_mul(pos[:], pos[:], inv[:, nt2:NT])

    # rhs tiles: zt[g][h][:, j*OWN:(j+1)*OWN] = core (2g+j)'s half-h block
    zt = [[None, None] for _ in range(NG)]
    for g in range(NG):
        for h in range(2):
            zt[g][h] = zt_pool.tile([P, GCOLS], DTIN, tag=f"zt{g}_{h}",
                                    name=f"zt{g}_{h}")
            for j in range(2):
                eng = nc.sync if (h + j) % 2 == 0 else nc.scalar
                eng.dma_start(zt[g][h][:, j * OWN:(j + 1) * OWN],
                              cc_out[2 * g + j, h])

    # ---------------- main loop ----------------
    def main_unit(g, m):
        ps = ps_pool.tile([P, GCOLS], F32, tag="ps", name="ps")
        nsub = GCOLS // 512
        for h in range(2):
            for ns in range(nsub):
                nc.tensor.matmul(
                    ps[:, ns * 512:(ns + 1) * 512],
                    lhsT=zt_own8[h][:, m * P:(m + 1) * P],
                    rhs=zt[g][h][:, ns * 512:(ns + 1) * 512],
                    start=(h == 0), stop=(h == 1))
        nc.scalar.activation(
            out=ps[:], in_=ps[:], func=AF.Exp, scale=INV_T,
            accum_out=rowparts[:, m * NG + g: m * NG + g + 1])

    for g in range(NG):
        for m in range(MT):
            main_unit(g, m)

    # ---------------- tail ----------------
    denom = fin_pool.tile([P, MT], F32, tag="denom")
    nc.vector.reduce_sum(
        out=denom[:], in_=rowparts[:].rearrange("p (m g) -> p m g", g=NG),
        axis=mybir.AxisListType.X)
    ln8 = fin_pool.tile([P, MT], F32, tag="ln8")
    nc.scalar.activation(out=ln8[:], in_=denom[:], func=AF.Ln, bias=negdiag[:])
    lnsum = fin_pool.tile([P, 1], F32, tag="lnsum")
    nc.vector.reduce_sum(out=lnsum[:], in_=ln8[:], axis=mybir.AxisListType.X)
    possum = fin_pool.tile([P, 1], F32, tag="possum")
    nc.vector.reduce_sum(out=possum[:], in_=pos[:], axis=mybir.AxisListType.X)
    partial = fin_pool.tile([P, 1], F32, tag="partial")
    # partial = lnsum - 2*INV_T*possum   (each pos appears for a z_i and a z_j row)
    nc.vector.tensor_scalar(
        out=partial[:], in0=possum[:], scalar1=-2.0 * INV_T, scalar2=lnsum[:],
        op0=ALU.mult, op1=ALU.add)
    nc.sync.dma_start(out_ap, partial[:])


_NC_CACHE = {}


def build_nc():
    if "nc" in _NC_CACHE:
        return _NC_CACHE["nc"]
    nc = bacc.Bacc("TRN2", target_bir_lowering=False, debug=False,
                   enable_asserts=False, num_devices=NCORES)
    x = nc.dram_tensor("x", (OWN, D), DTIN, kind="ExternalInput").ap()
    out = nc.dram_tensor("out", (P, 1), F32, kind="ExternalOutput").ap()
    with tile.TileContext(nc) as tc:
        with ExitStack() as ctx:
            _kernel_body(ctx, tc, out, x)
    nc.compile()
    _NC_CACHE["nc"] = nc
    return nc


def _make_xall(emb_i, emb_j):
    npin = mybir.dt.np(DTIN)
    xi = np.asarray(emb_i).astype(npin).reshape(NCORES, PER, D)
    xj = np.asarray(emb_j).astype(npin).reshape(NCORES, PER, D)
    return np.concatenate([xi, xj], axis=1)  # [8, 1024, 256]; [k] = core k shard


def make_in_maps(emb_i, emb_j):
    xall = _make_xall(emb_i, emb_j)
    return [{"x": xall[k]} for k in range(NCORES)]


def _build_fast_call(nc):
    """Cached-dispatch twin of bass_utils.run_bass_kernel_spmd's axon/PJRT
    exec step: same NEFF, same shard_map over cores 0-7, but the jitted
    callable is built once and reused, instead of a fresh closure (and a
    ~0.15s jax re-trace) per call."""
    import jax
    from concourse.bass2jax import (_bass_exec_p, install_neuronx_cc_hook,
                                    partition_id_tensor)
    from jax.sharding import Mesh, PartitionSpec
    from jax.experimental.shard_map import shard_map

    install_neuronx_cc_hook()
    partition_name = (nc.partition_id_tensor.name
                      if nc.partition_id_tensor else None)
    in_names, out_names, out_avals = [], [], []
    for alloc in nc.m.functions[0].allocations:
        if not isinstance(alloc, mybir.MemoryLocationSet):
            continue
        name = alloc.memorylocations[0].name
        if alloc.kind == "ExternalInput":
            if name != partition_name:
                in_names.append(name)
        elif alloc.kind == "ExternalOutput":
            out_names.append(name)
            shape = tuple(alloc.tensor_shape)
            out_avals.append(jax.core.ShapedArray(shape, mybir.dt.np(alloc.dtype)))
    assert in_names == ["x"] and out_names == ["out"]
    n_params = len(in_names)
    in_names.extend(out_names)
    if partition_name:
        in_names.append(partition_name)

    def _body(*args):
        operands = list(args)
        if partition_name:
            operands.append(partition_id_tensor())
        return tuple(_bass_exec_p.bind(
            *operands, out_avals=tuple(out_avals), in_names=tuple(in_names),
            out_names=tuple(out_names), lowering_input_output_aliases=(),
            sim_require_finite=True, sim_require_nnan=True, nc=nc))

    devices = jax.devices()[:NCORES]
    mesh = Mesh(np.asarray(devices), ("core",))
    specs = (PartitionSpec("core"),)
    sharded = jax.jit(
        shard_map(_body, mesh=mesh, in_specs=specs * (n_params + 1),
                  out_specs=specs, check_rep=False),
        donate_argnums=(n_params,), keep_unused=True)

    def call(xall):
        out, = sharded(xall.reshape(NCORES * OWN, D),
                       np.zeros((NCORES * P, 1), np.float32))
        return np.asarray(out).reshape(NCORES, P, 1)

    return call


def run(emb_i, emb_j, trace=False, **kw):
    nc = build_nc()
    fast = _NC_CACHE.get("fast_call")
    if trace or fast is None:
        res = bass_utils.run_bass_kernel_spmd(
            nc, make_in_maps(emb_i, emb_j), core_ids=list(range(NCORES)),
            trace=trace, **kw)
        partials = np.stack([r["out"] for r in res.results])  # [8,128,1]
        if fast is None:
            fast = _build_fast_call(nc)
            fast(_make_xall(emb_i, emb_j))  # warm jit trace + exec caches
            _NC_CACHE["fast_call"] = fast
    else:
        partials = fast(_make_xall(emb_i, emb_j))
        res = bass_utils.BassKernelResults(
            results=[{"out": partials[k]} for k in range(NCORES)],
            instructions_and_trace=None, profile_json=None, exec_time_ns=None)
    loss = np.float32(partials.astype(np.float64).sum() / ROWS)
    return loss, res


def kernel(emb_i, emb_j):
    loss, _ = run(emb_i, emb_j, trace=False)
    return np.asarray(loss, dtype=np.float32)


# revision 14
# speedup vs baseline: 1.0221x; 1.0221x over previous
"""Contrastive (NT-Xent) loss kernel for Trainium2, 8 NeuronCores SPMD.

Math (B=4096, D=256, T=0.5):
  z = l2norm(emb) rows; reps=[z_i; z_j] (8192 x 256); sim = reps @ reps.T
  denom_r = sum_{c != r} exp(sim[r,c]/T);  pos_m = z_i[m].z_j[m]
  loss = mean_r( ln(denom_r) - pos_r/T )

Distribution: core k receives ONLY its row shard x = [emb_i rows
[512k,512k+512); emb_j rows [512k,512k+512)] as fp16 (512KB/core instead
of a replicated 9MB) — H2D over the axon tunnel is the wall-clock
bottleneck, not device compute. Each core normalizes its 1024 rows,
transposes them to d-major fp16 tiles, and the 8 cores AllGather those
tiles HBM->HBM (512KB -> 4MB, ~15us on-chip, compute engines stay free).
The gathered column order is a core-major permutation of the reference
row order, which is harmless: the denominator is a permutation-invariant
row sum and the diagonal term is removed analytically (exp(2*||z||^2)=e^2).

Per-core main loop (8 m-tiles x 4 column groups of 2048):
  matmul fp16 -> PSUM fp32 [128,2048], ACT Exp(scale=2) in-place with
  accum_out -> per-(m,g) row partial sums; tail: ln(rowsum - e^2) minus
  4*sum(pos) -> per-partition partial [128,1] per core.
Host: loss = sum(partials)/(2B).  (gather/unshard = sum of shards)
"""

import numpy as np
from contextlib import ExitStack

import concourse.bass as bass
import concourse.tile as tile
from concourse import bacc, mybir
from concourse import bass_utils

B = 4096
D = 256
TEMP = 0.5
NCORES = 8
ROWS = 2 * B            # 8192 reps rows
PER = B // NCORES       # 512 rows of emb_i (and emb_j) per core
OWN = 2 * PER           # 1024 reps rows per core
P = 128
NG = 4                  # column groups
GCOLS = ROWS // NG      # 2048 columns per group
MT = OWN // P           # 8 m-tiles per core
NT = OWN // P           # 8 own row-tiles
F32 = mybir.dt.float32
DT = mybir.dt.float16   # matmul dtype
DTIN = mybir.dt.float8e4  # wire dtype (e4m3): halves H2D vs fp16; loss
                          # averages the ~6e-2 per-element quantization
                          # noise over 8192 rows x 8192 cols to ~1e-4
INV_T = 1.0 / TEMP      # 2.0
DIAG = float(np.exp(np.float32(INV_T), dtype=np.float32))  # exp(2*||z||^2), ||z||~1


def _kernel_body(ctx: ExitStack, tc: tile.TileContext, out_ap, x):
    nc = tc.nc
    AF = mybir.ActivationFunctionType
    ALU = mybir.AluOpType

    own_pool = ctx.enter_context(tc.tile_pool(name="own", bufs=1))
    zt_pool = ctx.enter_context(tc.tile_pool(name="zt", bufs=1))
    fin_pool = ctx.enter_context(tc.tile_pool(name="fin", bufs=1))
    ps_pool = ctx.enter_context(tc.tile_pool(name="ps", bufs=2, space="PSUM"))
    dram_pool = ctx.enter_context(tc.tile_pool(name="dram", bufs=1, space="DRAM"))

    rowparts = fin_pool.tile([P, MT * NG], F32, tag="rowparts")
    negdiag = fin_pool.tile([P, 1], F32, tag="negdiag")
    nc.gpsimd.memset(negdiag[:], -DIAG)

    # ---------------- own-block prologue ----------------
    # per-128-row-tile pipeline: load t -> cast t -> sq t -> reduce t, so
    # the norm chain streams behind the DMA instead of waiting for the
    # whole 256KB strided load (the collective trigger is downstream of
    # all of this, and every core's trigger time gates the rendezvous)
    # row mapping: SBUF (p, t) holds shard row 4p + t%4 of shard t//4
    # (i for t<4, j for t>=4). 1KB-contiguous per partition per DMA —
    # 4x fewer, 4x bigger descriptors than a row%128 layout. Any row
    # permutation is fine: the denominator sums over all columns, and
    # the i/j positive pairing (t <-> t+4) stays partition-aligned.
    nt2 = NT // 2
    own_x8 = own_pool.tile([P, NT, D], DTIN, tag="own_x8")  # [128,8,256]
    own_x = own_pool.tile([P, NT, D], DT, tag="own_x")
    sq3 = own_pool.tile([P, NT, D], F32, tag="sq3")
    sqs = own_pool.tile([P, NT], F32, tag="sqs")
    nc.sync.dma_start(own_x8[:, 0:nt2, :],
                      x[0:PER].rearrange("(p u) d -> p u d", u=nt2))
    nc.scalar.dma_start(own_x8[:, nt2:NT, :],
                        x[PER:].rearrange("(p u) d -> p u d", u=nt2))
    for t in range(NT):
        nc.vector.tensor_copy(own_x[:, t, :], own_x8[:, t, :])
        nc.vector.tensor_mul(sq3[:, t, :], own_x[:, t, :], own_x[:, t, :])
        nc.vector.reduce_sum(out=sqs[:, t:t + 1], in_=sq3[:, t, :],
                             axis=mybir.AxisListType.X)
    inv = own_pool.tile([P, NT], F32, tag="inv")
    nc.scalar.activation(out=inv[:], in_=sqs[:], func=AF.Ln)
    nc.scalar.activation(out=inv[:], in_=inv[:], func=AF.Exp, scale=-0.5)

    z_own = own_pool.tile([P, NT, D], DT, tag="z_own")
    for t in range(NT):
        nc.vector.tensor_scalar_mul(
            out=z_own[:, t, :], in0=own_x[:, t, :], scalar1=inv[:, t:t + 1])

    # transpose own rows to d-major: zt_own[h][d, col] with d in half h.
    # fp16 transposes (xbar needs 2-byte) alternating over both HWDGE
    # queues (SP + ACT), then a DVE cast to fp8 per half feeds the
    # collective with half the wire bytes; matmul also runs fp8 (2x PE).
    zt_own = [own_pool.tile([P, OWN], DT, tag=f"zt_own{h}", name=f"zt_own{h}")
              for h in range(2)]
    zt_own8 = [own_pool.tile([P, OWN], DTIN, tag=f"zt_own8{h}",
                             name=f"zt_own8{h}") for h in range(2)]
    cc_in = dram_pool.tile([2, P, OWN], DTIN, name="cc_in")
    for h in range(2):
        for t in range(NT):
            eng = nc.sync if t % 2 == 0 else nc.scalar
            eng.dma_start_transpose(
                out=zt_own[h][:, t * P:(t + 1) * P],
                in_=z_own[:, t, h * P:(h + 1) * P])
            # per-block fp8 cast pipelines behind its transpose instead of
            # one whole-half cast serializing after the last one
            nc.vector.tensor_copy(zt_own8[h][:, t * P:(t + 1) * P],
                                  zt_own[h][:, t * P:(t + 1) * P])
        nc.gpsimd.dma_start(cc_in[h], zt_own8[h][:])

    # ---------------- all-gather d-major z (fp8) ----------------
    cc_out = dram_pool.tile([NCORES, 2, P, OWN], DTIN, addr_space="Shared",
                            name="cc_out")
    nc.gpsimd.collective_compute(
        "AllGather", mybir.AluOpType.bypass,
        replica_groups=[list(range(NCORES))],
        ins=[cc_in.opt()], outs=[cc_out.opt()])

    # positives: pos_t = (x_i[t] . x_j[t]) * inv_i[t] * inv_j[t]
    # (issued after the collective trigger so DVE work hides in its shadow)
    nt2 = NT // 2
    pr3 = own_pool.tile([P, nt2, D], F32, tag="pr3")
    nc.vector.tensor_mul(pr3[:], own_x[:, 0:nt2, :], own_x[:, nt2:NT, :])
    pos = own_pool.tile([P, nt2], F32, tag="pos")
    nc.vector.reduce_sum(out=pos[:], in_=pr3[:], axis=mybir.AxisListType.X)
    nc.vector.tensor_mul(pos[:], pos[:], inv[:, 0:nt2])
    nc.vector.tensor_mul(pos[:], pos[:], inv[:, nt2:NT])

    # rhs tiles: zt[g][h][:, j*OWN:(j+1)*OWN] = core (2g+j)'s half-h block
    zt = [[None, None] for _ in range(NG)]
    for g in range(NG):
        for h in range(2):
            zt[g][h] = zt_pool.tile([P, GCOLS], DTIN, tag=f"zt{g}_{h}",
                                    name=f"zt{g}_{h}")
            for j in range(2):
                eng = nc.sync if (h + j) % 2 == 0 else nc.scalar
                eng.dma_start(zt[g][h][:, j * OWN:(j + 1) * OWN],
                              cc_out[2 * g + j, h])

    # ---------------- main loop ----------------
    def main_unit(g, m):
        ps = ps_pool.tile([P, GCOLS], F32, tag="ps", name="ps")
        nsub = GCOLS // 512
        for h in range(2):
            for ns in range(nsub):
                nc.tensor.matmul(
                    ps[:, ns * 512:(ns + 1) * 512],
                    lhsT=zt_own8[h][:, m * P:(m + 1) * P],
                    rhs=zt[g][h][:, ns * 512:(ns + 1) * 512],
                    start=(h == 0), stop=(h == 1))
        nc.scalar.activation(
            out=ps[:], in_=ps[:], func=AF.Exp, scale=INV_T,
            accum_out=rowparts[:, m * NG + g: m * NG + g + 1])

    # m-outer: the two lhsT tiles (h0/h1) of each m-tile are reused across
    # all 4 column groups back-to-back, and each m's denominator partial
    # reduces on the otherwise-idle DVE while later m's still matmul/exp
    denom = fin_pool.tile([P, MT], F32, tag="denom")
    for m in range(MT):
        for g in range(NG):
            main_unit(g, m)
        nc.vector.reduce_sum(
            out=denom[:, m:m + 1],
            in_=rowparts[:, m * NG:(m + 1) * NG], axis=mybir.AxisListType.X)

    # ---------------- tail ----------------
    ln8 = fin_pool.tile([P, MT], F32, tag="ln8")
    nc.scalar.activation(out=ln8[:], in_=denom[:], func=AF.Ln, bias=negdiag[:])
    lnsum = fin_pool.tile([P, 1], F32, tag="lnsum")
    nc.vector.reduce_sum(out=lnsum[:], in_=ln8[:], axis=mybir.AxisListType.X)
    possum = fin_pool.tile([P, 1], F32, tag="possum")
    nc.vector.reduce_sum(out=possum[:], in_=pos[:], axis=mybir.AxisListType.X)
    partial = fin_pool.tile([P, 1], F32, tag="partial")
    # partial = lnsum - 2*INV_T*possum   (each pos appears for a z_i and a z_j row)
    nc.vector.tensor_scalar(
        out=partial[:], in0=possum[:], scalar1=-2.0 * INV_T, scalar2=lnsum[:],
        op0=ALU.mult, op1=ALU.add)
    nc.sync.dma_start(out_ap, partial[:])


_NC_CACHE = {}


def build_nc():
    if "nc" in _NC_CACHE:
        return _NC_CACHE["nc"]
    nc = bacc.Bacc("TRN2", target_bir_lowering=False, debug=False,
                   enable_asserts=False, num_devices=NCORES)
    x = nc.dram_tensor("x", (OWN, D), DTIN, kind="ExternalInput").ap()
    out = nc.dram_tensor("out", (P, 1), F32, kind="ExternalOutput").ap()
    with tile.TileContext(nc) as tc:
        with ExitStack() as ctx:
            _kernel_body(ctx, tc, out, x)
    nc.compile()
    _NC_CACHE["nc"] = nc
    return nc


def _make_xall(emb_i, emb_j):
    npin = mybir.dt.np(DTIN)
    xi = np.asarray(emb_i).astype(npin).reshape(NCORES, PER, D)
    xj = np.asarray(emb_j).astype(npin).reshape(NCORES, PER, D)
    return np.concatenate([xi, xj], axis=1)  # [8, 1024, 256]; [k] = core k shard


def make_in_maps(emb_i, emb_j):
    xall = _make_xall(emb_i, emb_j)
    return [{"x": xall[k]} for k in range(NCORES)]


def _build_fast_call(nc):
    """Cached-dispatch twin of bass_utils.run_bass_kernel_spmd's axon/PJRT
    exec step: same NEFF, same shard_map over cores 0-7, but the jitted
    callable is built once and reused, instead of a fresh closure (and a
    ~0.15s jax re-trace) per call."""
    import jax
    from concourse.bass2jax import (_bass_exec_p, install_neuronx_cc_hook,
                                    partition_id_tensor)
    from jax.sharding import Mesh, PartitionSpec
    from jax.experimental.shard_map import shard_map

    install_neuronx_cc_hook()
    partition_name = (nc.partition_id_tensor.name
                      if nc.partition_id_tensor else None)
    in_names, out_names, out_avals = [], [], []
    for alloc in nc.m.functions[0].allocations:
        if not isinstance(alloc, mybir.MemoryLocationSet):
            continue
        name = alloc.memorylocations[0].name
        if alloc.kind == "ExternalInput":
            if name != partition_name:
                in_names.append(name)
        elif alloc.kind == "ExternalOutput":
            out_names.append(name)
            shape = tuple(alloc.tensor_shape)
            out_avals.append(jax.core.ShapedArray(shape, mybir.dt.np(alloc.dtype)))
    assert in_names == ["x"] and out_names == ["out"]
    n_params = len(in_names)
    in_names.extend(out_names)
    if partition_name:
        in_names.append(partition_name)

    def _body(*args):
        operands = list(args)
        if partition_name:
            operands.append(partition_id_tensor())
        return tuple(_bass_exec_p.bind(
            *operands, out_avals=tuple(out_avals), in_names=tuple(in_names),
            out_names=tuple(out_names), lowering_input_output_aliases=(),
            sim_require_finite=True, sim_require_nnan=True, nc=nc))

    devices = jax.devices()[:NCORES]
    mesh = Mesh(np.asarray(devices), ("core",))
    specs = (PartitionSpec("core"),)
    sharded = jax.jit(
        shard_map(_body, mesh=mesh, in_specs=specs * (n_params + 1),
                  out_specs=specs, check_rep=False),
        donate_argnums=(n_params,), keep_unused=True)

    def call(xall):
        out, = sharded(xall.reshape(NCORES * OWN, D),
                       np.zeros((NCORES * P, 1), np.float32))
        return np.asarray(out).reshape(NCORES, P, 1)

    return call


def run(emb_i, emb_j, trace=False, **kw):
    nc = build_nc()
    fast = _NC_CACHE.get("fast_call")
    if trace or fast is None:
        res = bass_utils.run_bass_kernel_spmd(
            nc, make_in_maps(emb_i, emb_j), core_ids=list(range(NCORES)),
            trace=trace, **kw)
        partials = np.stack([r["out"] for r in res.results])  # [8,128,1]
        if fast is None:
            fast = _build_fast_call(nc)
            fast(_make_xall(emb_i, emb_j))  # warm jit trace + exec caches
            _NC_CACHE["fast_call"] = fast
    else:
        partials = fast(_make_xall(emb_i, emb_j))
        res = bass_utils.BassKernelResults(
            results=[{"out": partials[k]} for k in range(NCORES)],
            instructions_and_trace=None, profile_json=None, exec_time_ns=None)
    loss = np.float32(partials.astype(np.float64).sum() / ROWS)
    return loss, res


def kernel(emb_i, emb_j):
    loss, _ = run(emb_i, emb_j, trace=False)
    return np.asarray(loss, dtype=np.float32)
